# revision 2
# baseline (speedup 1.0000x reference)
"""Trainium2 Bass kernel for nn_AttentionBlock (GroupNorm + 1x1-conv QKV
self-attention + residual), data-parallel over batch across 8 NeuronCores.

Per-core computation (one batch element, C=256 channels, N=64*64=4096 pixels):
  xn = GroupNorm(32, C)(x) * gn_scale + gn_bias      -- folded into QKV weights
  q/k/v = W @ xn + b                                 -- bf16 matmuls, fp32 psum
  S[n,m] = (q/16)[:,n] . k[:,m]; P = exp(S)          -- S^T kept [m_p, n_f]
  attn = (P^T v) / rowsum(P); out = x + Wo@attn + bo'

Layout notes: S^T is computed as [m(part), n(free)] so P^T tiles feed the
attention matmul directly (lhsT=v^T block, rhs=P^T block, contract m).
Rowsum of P uses a DVE strided reduce over m-blocks plus a GPSIMD
partition_all_reduce. The GroupNorm affine is folded algebraically into the
QKV weights (w' = w * a, bias' = w @ b + bias), so normalized x is never
materialized; the fp32 residual path streams x from DRAM.
"""

import sys

if "/opt/trn_rl_repo" not in sys.path:
    sys.path.insert(0, "/opt/trn_rl_repo")

from contextlib import ExitStack

import numpy as np

import concourse.tile as tile
from concourse import bacc, bass_isa, mybir
from concourse.bass_utils import run_bass_kernel_spmd

C = 256
H = W = 64
N = H * W  # 4096
GROUPS = 32
EPS = 1e-5
NCORES = 8
NCHUNK = 512  # n-tile width for the attention loop
NCH = N // NCHUNK  # 8
SCALE = 1.0 / 16.0  # 1/sqrt(C)

F32 = mybir.dt.float32
BF16 = mybir.dt.bfloat16
AF = mybir.ActivationFunctionType

# index of each packed per-channel vector in the "vecs" input
GN_SCALE, GN_BIAS, BQ, BK, BV, BO = range(6)


def _build():
    nc = bacc.Bacc("TRN2", target_bir_lowering=False, debug=False)

    x_d = nc.dram_tensor("x", [C, N], F32, kind="ExternalInput").ap()
    w_d = {
        name: nc.dram_tensor(f"{name}_t", [C, C], F32, kind="ExternalInput").ap()
        for name in ("wq", "wk", "wv", "wo")
    }
    vecs_d = nc.dram_tensor("vecs", [6, C], F32, kind="ExternalInput").ap()
    A_d = nc.dram_tensor("A", [128, 16], F32, kind="ExternalInput").ap()
    Bm_d = nc.dram_tensor("Bm", [16, 128], F32, kind="ExternalInput").ap()
    out_d = nc.dram_tensor("out", [C, N], F32, kind="ExternalOutput").ap()

    def osl(j):
        return slice(j * 128, (j + 1) * 128)

    with tile.TileContext(nc) as tc, ExitStack() as ctx:
        const = ctx.enter_context(tc.tile_pool(name="const", bufs=1))
        gtmp = ctx.enter_context(tc.tile_pool(name="gtmp", bufs=1))
        work = ctx.enter_context(tc.tile_pool(name="work", bufs=2))
        ptp = ctx.enter_context(tc.tile_pool(name="ptp", bufs=2))
        psum_st = ctx.enter_context(tc.tile_pool(name="pst", bufs=2, space="PSUM"))
        psum_mm = ctx.enter_context(tc.tile_pool(name="pmm", bufs=4, space="PSUM"))

        # ---- load inputs; stage x per c-block (fp32 for exact GN stats),
        # keep a bf16 copy resident for the matmuls ----
        w_sb = {}
        for name in ("wq", "wk", "wv", "wo"):
            t = const.tile([128, 2, C], F32, tag=name)
            for j in (0, 1):
                nc.sync.dma_start(out=t[:, j, :], in_=w_d[name][osl(j), :])
            w_sb[name] = t
        vecs = const.tile([128, 6, 2], F32)
        nc.sync.dma_start(out=vecs[:], in_=vecs_d.rearrange("v (j p) -> p v j", p=128))
        A_sb = const.tile([128, 16], F32)
        nc.sync.dma_start(out=A_sb[:], in_=A_d[:])
        Bm_sb = const.tile([16, 128], F32)
        nc.sync.dma_start(out=Bm_sb[:], in_=Bm_d[:])
        eps16 = const.tile([16, 1], F32)
        nc.vector.memset(eps16[:], EPS)

        x_bf = const.tile([128, 2, N], BF16)
        a_all = const.tile([128, 2], F32)
        b_all = const.tile([128, 2], F32)
        for j in (0, 1):
            stage = gtmp.tile([128, N], F32, tag="xstage")
            nc.sync.dma_start(out=stage[:], in_=x_d[osl(j), :])
            nc.vector.tensor_copy(out=x_bf[:, j, :], in_=stage[:])
            # ---- GroupNorm statistics -> per-partition affine a, b ----
            stats = gtmp.tile([128, 8, 6], F32)
            for s in range(8):
                nc.vector.bn_stats(
                    out=stats[:, s, :], in_=stage[:, s * 512 : (s + 1) * 512]
                )
            mv = gtmp.tile([128, 2], F32)
            nc.vector.bn_aggr(out=mv[:], in_=stats[:])
            # t2 = (mean_p, var_p + mean_p^2)
            t2 = gtmp.tile([128, 2], F32)
            nc.gpsimd.tensor_copy(out=t2[:, 0:1], in_=mv[:, 0:1])
            nc.vector.tensor_mul(t2[:, 1:2], mv[:, 0:1], mv[:, 0:1])
            nc.vector.tensor_add(t2[:, 1:2], t2[:, 1:2], mv[:, 1:2])
            # group stats: (1/8) * sum over the 8 partitions of each group
            ps_g = psum_mm.tile([16, 2], F32, tag="mm")
            nc.tensor.matmul(ps_g[:], lhsT=A_sb[:], rhs=t2[:], start=True, stop=True)
            sg = gtmp.tile([16, 2], F32)
            nc.vector.tensor_copy(out=sg[:], in_=ps_g[:])
            var16 = gtmp.tile([16, 1], F32)
            nc.vector.tensor_mul(var16[:], sg[:, 0:1], sg[:, 0:1])
            nc.vector.tensor_sub(var16[:], sg[:, 1:2], var16[:])
            nc.scalar.activation(
                out=var16[:], in_=var16[:], func=AF.Sqrt, bias=eps16[:], scale=1.0
            )
            rstd16 = gtmp.tile([16, 1], F32)
            nc.vector.reciprocal(out=rstd16[:], in_=var16[:])
            bc2 = gtmp.tile([16, 2], F32)
            nc.gpsimd.tensor_copy(out=bc2[:, 0:1], in_=sg[:, 0:1])
            nc.gpsimd.tensor_copy(out=bc2[:, 1:2], in_=rstd16[:])
            # broadcast group values back to the 128 partitions of this block
            ps_bc = psum_mm.tile([128, 2], F32, tag="mm")
            nc.tensor.matmul(ps_bc[:], lhsT=Bm_sb[:], rhs=bc2[:], start=True, stop=True)
            mr = gtmp.tile([128, 2], F32)
            nc.vector.tensor_copy(out=mr[:], in_=ps_bc[:])
            nc.vector.tensor_mul(
                a_all[:, j : j + 1], mr[:, 1:2], vecs[:, GN_SCALE, j : j + 1]
            )
            tmpb = gtmp.tile([128, 1], F32)
            nc.vector.tensor_mul(tmpb[:], mr[:, 0:1], a_all[:, j : j + 1])
            nc.vector.tensor_sub(
                b_all[:, j : j + 1], vecs[:, GN_BIAS, j : j + 1], tmpb[:]
            )

        # ---- fold GN affine into QKV weights and biases ----
        wsc = {}
        for name in ("wq", "wk", "wv"):
            t = const.tile([128, 2, C], BF16, tag=f"{name}s")
            for j in (0, 1):
                nc.vector.tensor_scalar_mul(
                    out=t[:, j, :], in0=w_sb[name][:, j, :], scalar1=a_all[:, j : j + 1]
                )
            wsc[name] = t
        wo_bf = const.tile([128, 2, C], BF16)
        for j in (0, 1):
            nc.vector.tensor_copy(out=wo_bf[:, j, :], in_=w_sb["wo"][:, j, :])
        fold = {}
        for name, bidx in (("wq", BQ), ("wk", BK), ("wv", BV)):
            fb = const.tile([128, 2], F32, tag=f"{name}fb")
            for ob in (0, 1):
                ps = psum_mm.tile([128, 1], F32, tag="mm")
                for j in (0, 1):
                    nc.tensor.matmul(
                        ps[:],
                        lhsT=w_sb[name][:, j, osl(ob)],
                        rhs=b_all[:, j : j + 1],
                        start=(j == 0),
                        stop=(j == 1),
                    )
                nc.scalar.activation(
                    out=fb[:, ob : ob + 1],
                    in_=ps[:],
                    func=AF.Identity,
                    bias=vecs[:, bidx, ob : ob + 1],
                    scale=1.0,
                )
            fold[name] = fb
        bop = const.tile([128, 2], F32)
        for ob in (0, 1):
            ps = psum_mm.tile([128, 1], F32, tag="mm")
            for j in (0, 1):
                nc.tensor.matmul(
                    ps[:],
                    lhsT=w_sb["wo"][:, j, osl(ob)],
                    rhs=fold["wv"][:, j : j + 1],
                    start=(j == 0),
                    stop=(j == 1),
                )
            nc.scalar.activation(
                out=bop[:, ob : ob + 1],
                in_=ps[:],
                func=AF.Identity,
                bias=vecs[:, BO, ob : ob + 1],
                scale=1.0,
            )

        # ---- QKV projections (bf16 in, fp32 psum, bf16 out) ----
        q_sb = const.tile([128, 2, N], BF16)
        k_sb = const.tile([128, 2, N], BF16)
        v_sb = const.tile([128, 32, C], BF16)
        for name, dest, scale in (("wq", q_sb, SCALE), ("wk", k_sb, 1.0)):
            for ob in (0, 1):
                for i in range(NCH):
                    nsl = slice(i * NCHUNK, (i + 1) * NCHUNK)
                    ps = psum_mm.tile([128, NCHUNK], F32, tag="mm")
                    for j in (0, 1):
                        nc.tensor.matmul(
                            ps[:],
                            lhsT=wsc[name][:, j, osl(ob)],
                            rhs=x_bf[:, j, nsl],
                            start=(j == 0),
                            stop=(j == 1),
                        )
                    nc.scalar.activation(
                        out=dest[:, ob, nsl],
                        in_=ps[:],
                        func=AF.Identity,
                        bias=fold[name][:, ob : ob + 1],
                        scale=scale,
                    )
        # v^T: [m(part-block), c(free)] so it can serve as attention lhsT
        for mp in range(16):
            ps = psum_mm.tile([128, 2, C], F32, tag="mm")
            for s2 in (0, 1):
                mb = 2 * mp + s2
                for j in (0, 1):
                    nc.tensor.matmul(
                        ps[:, s2, :],
                        lhsT=x_bf[:, j, mb * 128 : (mb + 1) * 128],
                        rhs=wsc["wv"][:, j, :],
                        start=(j == 0),
                        stop=(j == 1),
                    )
            nc.scalar.copy(out=v_sb[:, 2 * mp : 2 * mp + 2, :], in_=ps[:])

        # ---- attention, one 512-wide n-chunk at a time ----
        def scores_chunk(i):
            nsl = slice(i * NCHUNK, (i + 1) * NCHUNK)
            pt = ptp.tile([128, 32, NCHUNK], BF16, tag="pt")
            for mp in range(16):
                ps = psum_st.tile([128, 2, NCHUNK], F32, tag="st")
                for s2 in (0, 1):
                    mb = 2 * mp + s2
                    for j in (0, 1):
                        nc.tensor.matmul(
                            ps[:, s2, :],
                            lhsT=k_sb[:, j, mb * 128 : (mb + 1) * 128],
                            rhs=q_sb[:, j, nsl],
                            start=(j == 0),
                            stop=(j == 1),
                        )
                nc.scalar.activation(
                    out=pt[:, 2 * mp : 2 * mp + 2, :], in_=ps[:], func=AF.Exp
                )
            return pt

        def post_chunk(i, pt):
            nsl = slice(i * NCHUNK, (i + 1) * NCHUNK)
            # softmax denominator: sum P over all m = (reduce over m-blocks,
            # then all-reduce over partitions)
            rs = work.tile([128, NCHUNK], F32, tag="rs")
            nc.vector.reduce_sum(
                out=rs[:], in_=pt[:].transpose([0, 2, 1]), axis=mybir.AxisListType.X
            )
            z = work.tile([128, NCHUNK], F32, tag="z")
            nc.gpsimd.partition_all_reduce(
                z[:], rs[:], channels=128, reduce_op=bass_isa.ReduceOp.add
            )
            rcp = work.tile([128, NCHUNK], F32, tag="rcp")
            nc.vector.reciprocal(out=rcp[:], in_=z[:])
            attn = work.tile([128, 2, NCHUNK], BF16, tag="attn")
            for cb in (0, 1):
                ps = psum_mm.tile([128, NCHUNK], F32, tag="mm")
                for mb in range(32):
                    nc.tensor.matmul(
                        ps[:],
                        lhsT=v_sb[:, mb, osl(cb)],
                        rhs=pt[:, mb, :],
                        start=(mb == 0),
                        stop=(mb == 31),
                    )
                nc.vector.tensor_mul(attn[:, cb, :], ps[:], rcp[:])
            xres = work.tile([128, 2, NCHUNK], F32, tag="xres")
            for ob in (0, 1):
                nc.sync.dma_start(out=xres[:, ob, :], in_=x_d[osl(ob), nsl])
            ot = work.tile([128, 2, NCHUNK], F32, tag="ot")
            for ob in (0, 1):
                ps = psum_mm.tile([128, NCHUNK], F32, tag="mm")
                for j in (0, 1):
                    nc.tensor.matmul(
                        ps[:],
                        lhsT=wo_bf[:, j, osl(ob)],
                        rhs=attn[:, j, :],
                        start=(j == 0),
                        stop=(j == 1),
                    )
                nc.scalar.activation(
                    out=ot[:, ob, :],
                    in_=ps[:],
                    func=AF.Identity,
                    bias=bop[:, ob : ob + 1],
                    scale=1.0,
                )
            nc.vector.tensor_add(ot[:], ot[:], xres[:])
            for ob in (0, 1):
                nc.sync.dma_start(out=out_d[osl(ob), nsl], in_=ot[:, ob, :])

        prev = None
        for i in range(NCH):
            pt = scores_chunk(i)
            if prev is not None:
                post_chunk(i - 1, prev)
            prev = pt
        post_chunk(NCH - 1, prev)

    nc.compile()
    return nc


_NC_CACHE = None


def _get_nc():
    global _NC_CACHE
    if _NC_CACHE is None:
        _NC_CACHE = _build()
    return _NC_CACHE


def _host_inputs(x, gn_scale, gn_bias, wq, bq, wk, bk, wv, bv, wo, bo):
    x = np.asarray(x, dtype=np.float32)
    b = x.shape[0]
    assert x.shape == (b, C, H, W) and b == NCORES
    xs = np.ascontiguousarray(x.reshape(b, C, N))
    vecs = np.ascontiguousarray(
        np.stack(
            [
                np.asarray(v, np.float32).reshape(C)
                for v in (gn_scale, gn_bias, bq, bk, bv, bo)
            ]
        )
    )
    wt = {
        name: np.ascontiguousarray(np.asarray(w, np.float32).T)
        for name, w in (("wq", wq), ("wk", wk), ("wv", wv), ("wo", wo))
    }
    A = np.zeros((128, 16), np.float32)
    A[np.arange(128), np.arange(128) // 8] = 1.0 / 8.0
    Bm = np.zeros((16, 128), np.float32)
    Bm[np.arange(128) // 8, np.arange(128)] = 1.0
    return [
        {
            "x": xs[i],
            "wq_t": wt["wq"],
            "wk_t": wt["wk"],
            "wv_t": wt["wv"],
            "wo_t": wt["wo"],
            "vecs": vecs,
            "A": A,
            "Bm": Bm,
        }
        for i in range(NCORES)
    ]


def kernel(x, gn_scale, gn_bias, wq, bq, wk, bk, wv, bv, wo, bo):
    in_maps = _host_inputs(x, gn_scale, gn_bias, wq, bq, wk, bk, wv, bv, wo, bo)
    nc = _get_nc()
    res = run_bass_kernel_spmd(nc, in_maps, core_ids=list(range(NCORES)))
    out = np.stack([res.results[i]["out"] for i in range(NCORES)])
    return np.ascontiguousarray(out.reshape(NCORES, C, H, W))


# revision 4
# speedup vs baseline: 1.3817x; 1.3817x over previous
"""Trainium2 Bass kernel for nn_AttentionBlock (GroupNorm + 1x1-conv QKV
self-attention + residual), data-parallel over batch across 8 NeuronCores.

Per-core computation (one batch element, C=256 channels, N=64*64=4096 pixels):
  xn = GroupNorm(32, C)(x) * gn_scale + gn_bias      -- folded into QKV weights
  q/k/v = W @ xn + b                                 -- bf16 matmuls, fp32 psum
  S[n,m] = (q/16)[:,n] . k[:,m]; P = exp(S)          -- S^T kept [m_p, n_f]
  attn = (P^T v) / rowsum(P); out = x + Wo@attn + bo'

Layout notes: S^T is computed as [m(part), n(free)] so P^T tiles feed the
attention matmul directly (lhsT=v^T block, rhs=P^T block, contract m).
Rowsum of P uses a DVE strided reduce over m-blocks plus a GPSIMD
partition_all_reduce. The GroupNorm affine is folded algebraically into the
QKV weights (w' = w * a, bias' = w @ b + bias), so normalized x is never
materialized; the fp32 residual path streams x from DRAM.
"""

import sys

if "/opt/trn_rl_repo" not in sys.path:
    sys.path.insert(0, "/opt/trn_rl_repo")

from contextlib import ExitStack

import numpy as np

import concourse.tile as tile
from concourse import bacc, bass_isa, mybir
from concourse.bass_utils import run_bass_kernel_spmd

C = 256
H = W = 64
N = H * W  # 4096
GROUPS = 32
EPS = 1e-5
NCORES = 8
NCHUNK = 512  # n-tile width for the attention loop
NCH = N // NCHUNK  # 8
SCALE = 1.0 / 16.0  # 1/sqrt(C)

F32 = mybir.dt.float32
BF16 = mybir.dt.bfloat16
AF = mybir.ActivationFunctionType

# index of each packed per-channel vector in the "vecs" input
GN_SCALE, GN_BIAS, BQ, BK, BV, BO = range(6)


def _build():
    nc = bacc.Bacc("TRN2", target_bir_lowering=False, debug=False)

    x_d = nc.dram_tensor("x", [C, N], F32, kind="ExternalInput").ap()
    w_d = {
        name: nc.dram_tensor(f"{name}_t", [C, C], F32, kind="ExternalInput").ap()
        for name in ("wq", "wk", "wv", "wo")
    }
    vecs_d = nc.dram_tensor("vecs", [6, C], F32, kind="ExternalInput").ap()
    A_d = nc.dram_tensor("A", [128, 16], F32, kind="ExternalInput").ap()
    Bm_d = nc.dram_tensor("Bm", [16, 128], F32, kind="ExternalInput").ap()
    out_d = nc.dram_tensor("out", [C, N], F32, kind="ExternalOutput").ap()

    def osl(j):
        return slice(j * 128, (j + 1) * 128)

    with tile.TileContext(nc) as tc, ExitStack() as ctx:
        const = ctx.enter_context(tc.tile_pool(name="const", bufs=1))
        gtmp = ctx.enter_context(tc.tile_pool(name="gtmp", bufs=1))
        work = ctx.enter_context(tc.tile_pool(name="work", bufs=2))
        ptp = ctx.enter_context(tc.tile_pool(name="ptp", bufs=2))
        psum_st = ctx.enter_context(tc.tile_pool(name="pst", bufs=2, space="PSUM"))
        psum_mm = ctx.enter_context(tc.tile_pool(name="pmm", bufs=4, space="PSUM"))

        # ---- load inputs; stage x per c-block (fp32 for exact GN stats),
        # keep a bf16 copy resident for the matmuls ----
        w_sb = {}
        for name in ("wq", "wk", "wv", "wo"):
            t = const.tile([128, 2, C], F32, tag=name)
            for j in (0, 1):
                nc.sync.dma_start(out=t[:, j, :], in_=w_d[name][osl(j), :])
            w_sb[name] = t
        vecs = const.tile([128, 6, 2], F32)
        nc.sync.dma_start(out=vecs[:], in_=vecs_d.rearrange("v (j p) -> p v j", p=128))
        A_sb = const.tile([128, 16], F32)
        nc.sync.dma_start(out=A_sb[:], in_=A_d[:])
        Bm_sb = const.tile([16, 128], F32)
        nc.sync.dma_start(out=Bm_sb[:], in_=Bm_d[:])
        eps16 = const.tile([16, 1], F32)
        nc.vector.memset(eps16[:], EPS)
        ones_bf = const.tile([128, 128], BF16)
        nc.vector.memset(ones_bf[:], 1.0)

        x_bf = const.tile([128, 2, N], BF16)
        a_all = const.tile([128, 2], F32)
        b_all = const.tile([128, 2], F32)
        for j in (0, 1):
            stage = gtmp.tile([128, N], F32, tag="xstage")
            nc.sync.dma_start(out=stage[:], in_=x_d[osl(j), :])
            nc.vector.tensor_copy(out=x_bf[:, j, :], in_=stage[:])
            # ---- GroupNorm statistics -> per-partition affine a, b ----
            stats = gtmp.tile([128, 8, 6], F32)
            for s in range(8):
                nc.vector.bn_stats(
                    out=stats[:, s, :], in_=stage[:, s * 512 : (s + 1) * 512]
                )
            mv = gtmp.tile([128, 2], F32)
            nc.vector.bn_aggr(out=mv[:], in_=stats[:])
            # t2 = (mean_p, var_p + mean_p^2)
            t2 = gtmp.tile([128, 2], F32)
            nc.gpsimd.tensor_copy(out=t2[:, 0:1], in_=mv[:, 0:1])
            nc.vector.tensor_mul(t2[:, 1:2], mv[:, 0:1], mv[:, 0:1])
            nc.vector.tensor_add(t2[:, 1:2], t2[:, 1:2], mv[:, 1:2])
            # group stats: (1/8) * sum over the 8 partitions of each group
            ps_g = psum_mm.tile([16, 2], F32, tag="mm")
            nc.tensor.matmul(ps_g[:], lhsT=A_sb[:], rhs=t2[:], start=True, stop=True)
            sg = gtmp.tile([16, 2], F32)
            nc.vector.tensor_copy(out=sg[:], in_=ps_g[:])
            var16 = gtmp.tile([16, 1], F32)
            nc.vector.tensor_mul(var16[:], sg[:, 0:1], sg[:, 0:1])
            nc.vector.tensor_sub(var16[:], sg[:, 1:2], var16[:])
            nc.scalar.activation(
                out=var16[:], in_=var16[:], func=AF.Sqrt, bias=eps16[:], scale=1.0
            )
            rstd16 = gtmp.tile([16, 1], F32)
            nc.vector.reciprocal(out=rstd16[:], in_=var16[:])
            bc2 = gtmp.tile([16, 2], F32)
            nc.gpsimd.tensor_copy(out=bc2[:, 0:1], in_=sg[:, 0:1])
            nc.gpsimd.tensor_copy(out=bc2[:, 1:2], in_=rstd16[:])
            # broadcast group values back to the 128 partitions of this block
            ps_bc = psum_mm.tile([128, 2], F32, tag="mm")
            nc.tensor.matmul(ps_bc[:], lhsT=Bm_sb[:], rhs=bc2[:], start=True, stop=True)
            mr = gtmp.tile([128, 2], F32)
            nc.vector.tensor_copy(out=mr[:], in_=ps_bc[:])
            nc.vector.tensor_mul(
                a_all[:, j : j + 1], mr[:, 1:2], vecs[:, GN_SCALE, j : j + 1]
            )
            tmpb = gtmp.tile([128, 1], F32)
            nc.vector.tensor_mul(tmpb[:], mr[:, 0:1], a_all[:, j : j + 1])
            nc.vector.tensor_sub(
                b_all[:, j : j + 1], vecs[:, GN_BIAS, j : j + 1], tmpb[:]
            )

        # ---- fold GN affine into QKV weights and biases ----
        wsc = {}
        for name in ("wq", "wk", "wv"):
            t = const.tile([128, 2, C], BF16, tag=f"{name}s")
            for j in (0, 1):
                nc.vector.tensor_scalar_mul(
                    out=t[:, j, :], in0=w_sb[name][:, j, :], scalar1=a_all[:, j : j + 1]
                )
            wsc[name] = t
        wo_bf = const.tile([128, 2, C], BF16)
        for j in (0, 1):
            nc.vector.tensor_copy(out=wo_bf[:, j, :], in_=w_sb["wo"][:, j, :])
        fold = {}
        for name, bidx in (("wq", BQ), ("wk", BK), ("wv", BV)):
            fb = const.tile([128, 2], F32, tag=f"{name}fb")
            for ob in (0, 1):
                ps = psum_mm.tile([128, 1], F32, tag="mm")
                for j in (0, 1):
                    nc.tensor.matmul(
                        ps[:],
                        lhsT=w_sb[name][:, j, osl(ob)],
                        rhs=b_all[:, j : j + 1],
                        start=(j == 0),
                        stop=(j == 1),
                    )
                nc.scalar.activation(
                    out=fb[:, ob : ob + 1],
                    in_=ps[:],
                    func=AF.Identity,
                    bias=vecs[:, bidx, ob : ob + 1],
                    scale=1.0,
                )
            fold[name] = fb
        bop = const.tile([128, 2], F32)
        for ob in (0, 1):
            ps = psum_mm.tile([128, 1], F32, tag="mm")
            for j in (0, 1):
                nc.tensor.matmul(
                    ps[:],
                    lhsT=w_sb["wo"][:, j, osl(ob)],
                    rhs=fold["wv"][:, j : j + 1],
                    start=(j == 0),
                    stop=(j == 1),
                )
            nc.scalar.activation(
                out=bop[:, ob : ob + 1],
                in_=ps[:],
                func=AF.Identity,
                bias=vecs[:, BO, ob : ob + 1],
                scale=1.0,
            )

        # ---- QKV projections (bf16 in, fp32 psum, bf16 out) ----
        q_sb = const.tile([128, 2, N], BF16)
        k_sb = const.tile([128, 2, N], BF16)
        v_sb = const.tile([128, 32, C], BF16)
        for name, dest, scale in (("wq", q_sb, SCALE), ("wk", k_sb, 1.0)):
            for ob in (0, 1):
                for i in range(NCH):
                    nsl = slice(i * NCHUNK, (i + 1) * NCHUNK)
                    ps = psum_mm.tile([128, NCHUNK], F32, tag="mm")
                    for j in (0, 1):
                        nc.tensor.matmul(
                            ps[:],
                            lhsT=wsc[name][:, j, osl(ob)],
                            rhs=x_bf[:, j, nsl],
                            start=(j == 0),
                            stop=(j == 1),
                        )
                    nc.scalar.activation(
                        out=dest[:, ob, nsl],
                        in_=ps[:],
                        func=AF.Identity,
                        bias=fold[name][:, ob : ob + 1],
                        scale=scale,
                    )
        # v^T: [m(part-block), c(free)] so it can serve as attention lhsT
        for mp in range(16):
            ps = psum_mm.tile([128, 2, C], F32, tag="mm")
            for s2 in (0, 1):
                mb = 2 * mp + s2
                for j in (0, 1):
                    nc.tensor.matmul(
                        ps[:, s2, :],
                        lhsT=x_bf[:, j, mb * 128 : (mb + 1) * 128],
                        rhs=wsc["wv"][:, j, :],
                        start=(j == 0),
                        stop=(j == 1),
                    )
            nc.scalar.copy(out=v_sb[:, 2 * mp : 2 * mp + 2, :], in_=ps[:])

        # ---- attention, one 512-wide n-chunk at a time ----
        def scores_chunk(i):
            nsl = slice(i * NCHUNK, (i + 1) * NCHUNK)
            pt = ptp.tile([128, 32, NCHUNK], BF16, tag="pt")
            for mp in range(16):
                ps = psum_st.tile([128, 2, NCHUNK], F32, tag="st")
                for s2 in (0, 1):
                    mb = 2 * mp + s2
                    for j in (0, 1):
                        nc.tensor.matmul(
                            ps[:, s2, :],
                            lhsT=k_sb[:, j, mb * 128 : (mb + 1) * 128],
                            rhs=q_sb[:, j, nsl],
                            start=(j == 0),
                            stop=(j == 1),
                        )
                nc.scalar.activation(
                    out=pt[:, 2 * mp : 2 * mp + 2, :], in_=ps[:], func=AF.Exp
                )
            return pt

        def post_chunk(i, pt):
            nsl = slice(i * NCHUNK, (i + 1) * NCHUNK)
            # softmax denominator on the PE: z[p, n] = sum over all m of
            # P[m, n] (ones lhsT stays stationary; every out partition gets
            # the full sum, so this is reduce + broadcast in one pass)
            zps = psum_mm.tile([128, NCHUNK], F32, tag="mm")
            for mb in range(32):
                nc.tensor.matmul(
                    zps[:],
                    lhsT=ones_bf[:],
                    rhs=pt[:, mb, :],
                    start=(mb == 0),
                    stop=(mb == 31),
                )
            rcp = work.tile([128, NCHUNK], F32, tag="rcp")
            nc.vector.reciprocal(out=rcp[:], in_=zps[:])
            attn = work.tile([128, 2, NCHUNK], BF16, tag="attn")
            for cb in (0, 1):
                ps = psum_mm.tile([128, NCHUNK], F32, tag="mm")
                for mb in range(32):
                    nc.tensor.matmul(
                        ps[:],
                        lhsT=v_sb[:, mb, osl(cb)],
                        rhs=pt[:, mb, :],
                        start=(mb == 0),
                        stop=(mb == 31),
                    )
                nc.vector.tensor_mul(attn[:, cb, :], ps[:], rcp[:])
            xres = work.tile([128, 2, NCHUNK], F32, tag="xres")
            for ob in (0, 1):
                nc.sync.dma_start(out=xres[:, ob, :], in_=x_d[osl(ob), nsl])
            ot = work.tile([128, 2, NCHUNK], F32, tag="ot")
            for ob in (0, 1):
                ps = psum_mm.tile([128, NCHUNK], F32, tag="mm")
                for j in (0, 1):
                    nc.tensor.matmul(
                        ps[:],
                        lhsT=wo_bf[:, j, osl(ob)],
                        rhs=attn[:, j, :],
                        start=(j == 0),
                        stop=(j == 1),
                    )
                nc.scalar.activation(
                    out=ot[:, ob, :],
                    in_=ps[:],
                    func=AF.Identity,
                    bias=bop[:, ob : ob + 1],
                    scale=1.0,
                )
            nc.vector.tensor_add(ot[:], ot[:], xres[:])
            for ob in (0, 1):
                nc.sync.dma_start(out=out_d[osl(ob), nsl], in_=ot[:, ob, :])

        prev = None
        for i in range(NCH):
            pt = scores_chunk(i)
            if prev is not None:
                post_chunk(i - 1, prev)
            prev = pt
        post_chunk(NCH - 1, prev)

    nc.compile()
    return nc


_NC_CACHE = None


def _get_nc():
    global _NC_CACHE
    if _NC_CACHE is None:
        _NC_CACHE = _build()
    return _NC_CACHE


def _host_inputs(x, gn_scale, gn_bias, wq, bq, wk, bk, wv, bv, wo, bo):
    x = np.asarray(x, dtype=np.float32)
    b = x.shape[0]
    assert x.shape == (b, C, H, W) and b == NCORES
    xs = np.ascontiguousarray(x.reshape(b, C, N))
    vecs = np.ascontiguousarray(
        np.stack(
            [
                np.asarray(v, np.float32).reshape(C)
                for v in (gn_scale, gn_bias, bq, bk, bv, bo)
            ]
        )
    )
    wt = {
        name: np.ascontiguousarray(np.asarray(w, np.float32).T)
        for name, w in (("wq", wq), ("wk", wk), ("wv", wv), ("wo", wo))
    }
    A = np.zeros((128, 16), np.float32)
    A[np.arange(128), np.arange(128) // 8] = 1.0 / 8.0
    Bm = np.zeros((16, 128), np.float32)
    Bm[np.arange(128) // 8, np.arange(128)] = 1.0
    return [
        {
            "x": xs[i],
            "wq_t": wt["wq"],
            "wk_t": wt["wk"],
            "wv_t": wt["wv"],
            "wo_t": wt["wo"],
            "vecs": vecs,
            "A": A,
            "Bm": Bm,
        }
        for i in range(NCORES)
    ]


def kernel(x, gn_scale, gn_bias, wq, bq, wk, bk, wv, bv, wo, bo):
    in_maps = _host_inputs(x, gn_scale, gn_bias, wq, bq, wk, bk, wv, bv, wo, bo)
    nc = _get_nc()
    res = run_bass_kernel_spmd(nc, in_maps, core_ids=list(range(NCORES)))
    out = np.stack([res.results[i]["out"] for i in range(NCORES)])
    return np.ascontiguousarray(out.reshape(NCORES, C, H, W))


# revision 7
# speedup vs baseline: 1.5748x; 1.1398x over previous
"""Trainium2 Bass kernel for nn_AttentionBlock (GroupNorm + 1x1-conv QKV
self-attention + residual), data-parallel over batch across 8 NeuronCores.

Per-core computation (one batch element, C=256 channels, N=64*64=4096 pixels):
  xn = GroupNorm(32, C)(x) * gn_scale + gn_bias      -- folded into QKV weights
  q/k/v = W @ xn + b                                 -- bf16 matmuls, fp32 psum
  S[n,m] = (q/16)[:,n] . k[:,m]; P = exp(S)          -- S^T kept [m_p, n_f]
  attn = (P^T v) / rowsum(P); out = x + Wo@attn + bo'

Layout notes: S^T is computed as [m(part), n(free)] so P^T tiles feed the
attention matmul directly (lhsT=v^T block, rhs=P^T block, contract m).
Rowsum of P uses a DVE strided reduce over m-blocks plus a GPSIMD
partition_all_reduce. The GroupNorm affine is folded algebraically into the
QKV weights (w' = w * a, bias' = w @ b + bias), so normalized x is never
materialized; the fp32 residual path streams x from DRAM.
"""

import sys

if "/opt/trn_rl_repo" not in sys.path:
    sys.path.insert(0, "/opt/trn_rl_repo")

from contextlib import ExitStack

import numpy as np

import concourse.tile as tile
from concourse import bacc, bass_isa, mybir
from concourse.bass_utils import run_bass_kernel_spmd

C = 256
H = W = 64
N = H * W  # 4096
GROUPS = 32
EPS = 1e-5
NCORES = 8
NCHUNK = 512  # n-tile width for the attention loop
NCH = N // NCHUNK  # 8
SCALE = 1.0 / 16.0  # 1/sqrt(C)

F32 = mybir.dt.float32
BF16 = mybir.dt.bfloat16
AF = mybir.ActivationFunctionType

# index of each packed per-channel vector in the "vecs" input
GN_SCALE, GN_BIAS, BQ, BK, BV, BO = range(6)


def _build():
    nc = bacc.Bacc("TRN2", target_bir_lowering=False, debug=False)

    x_d = nc.dram_tensor("x", [C, N], F32, kind="ExternalInput").ap()
    w_d = {
        name: nc.dram_tensor(f"{name}_t", [C, C], F32, kind="ExternalInput").ap()
        for name in ("wq", "wk", "wv", "wo")
    }
    vecs_d = nc.dram_tensor("vecs", [6, C], F32, kind="ExternalInput").ap()
    A_d = nc.dram_tensor("A", [128, 16], F32, kind="ExternalInput").ap()
    Bm_d = nc.dram_tensor("Bm", [16, 128], F32, kind="ExternalInput").ap()
    out_d = nc.dram_tensor("out", [C, N], F32, kind="ExternalOutput").ap()

    def osl(j):
        return slice(j * 128, (j + 1) * 128)

    with tile.TileContext(nc) as tc, ExitStack() as ctx:
        const = ctx.enter_context(tc.tile_pool(name="const", bufs=1))
        gtmp = ctx.enter_context(tc.tile_pool(name="gtmp", bufs=1))
        work = ctx.enter_context(tc.tile_pool(name="work", bufs=2))
        ptp = ctx.enter_context(tc.tile_pool(name="ptp", bufs=2))
        psum_st = ctx.enter_context(tc.tile_pool(name="pst", bufs=2, space="PSUM"))
        psum_mm = ctx.enter_context(tc.tile_pool(name="pmm", bufs=4, space="PSUM"))

        # ---- load inputs; stage x per c-block (fp32 for exact GN stats),
        # keep a bf16 copy resident for the matmuls ----
        w_sb = {}
        for name in ("wq", "wk", "wv", "wo"):
            t = const.tile([128, 2, C], F32, tag=name)
            for j in (0, 1):
                nc.sync.dma_start(out=t[:, j, :], in_=w_d[name][osl(j), :])
            w_sb[name] = t
        vecs = const.tile([128, 6, 2], F32)
        nc.sync.dma_start(out=vecs[:], in_=vecs_d.rearrange("v (j p) -> p v j", p=128))
        A_sb = const.tile([128, 16], F32)
        nc.sync.dma_start(out=A_sb[:], in_=A_d[:])
        Bm_sb = const.tile([16, 128], F32)
        nc.sync.dma_start(out=Bm_sb[:], in_=Bm_d[:])
        eps16 = const.tile([16, 1], F32)
        nc.vector.memset(eps16[:], EPS)
        ones_f = const.tile([128, 128], F32)
        nc.vector.memset(ones_f[:], 1.0)

        x_bf = const.tile([128, 2, N], BF16)
        a_all = const.tile([128, 2], F32)
        b_all = const.tile([128, 2], F32)
        for j in (0, 1):
            stage = gtmp.tile([128, N], F32, tag="stage16")
            for s in range(4):
                ssl = slice(s * 1024, (s + 1) * 1024)
                nc.sync.dma_start(out=stage[:, ssl], in_=x_d[osl(j), ssl])
            nc.vector.tensor_copy(out=x_bf[:, j, :], in_=stage[:])
            # ---- GroupNorm statistics -> per-partition affine a, b ----
            stats = gtmp.tile([128, 8, 6], F32)
            for s in range(8):
                nc.vector.bn_stats(
                    out=stats[:, s, :], in_=stage[:, s * 512 : (s + 1) * 512]
                )
            mv = gtmp.tile([128, 2], F32)
            nc.vector.bn_aggr(out=mv[:], in_=stats[:])
            # t2 = (mean_p, var_p + mean_p^2)
            t2 = gtmp.tile([128, 2], F32)
            nc.gpsimd.tensor_copy(out=t2[:, 0:1], in_=mv[:, 0:1])
            nc.vector.tensor_mul(t2[:, 1:2], mv[:, 0:1], mv[:, 0:1])
            nc.vector.tensor_add(t2[:, 1:2], t2[:, 1:2], mv[:, 1:2])
            # group stats: (1/8) * sum over the 8 partitions of each group
            ps_g = psum_mm.tile([16, 2], F32, tag="mm")
            nc.tensor.matmul(ps_g[:], lhsT=A_sb[:], rhs=t2[:], start=True, stop=True)
            sg = gtmp.tile([16, 2], F32)
            nc.vector.tensor_copy(out=sg[:], in_=ps_g[:])
            var16 = gtmp.tile([16, 1], F32)
            nc.vector.tensor_mul(var16[:], sg[:, 0:1], sg[:, 0:1])
            nc.vector.tensor_sub(var16[:], sg[:, 1:2], var16[:])
            nc.scalar.activation(
                out=var16[:], in_=var16[:], func=AF.Sqrt, bias=eps16[:], scale=1.0
            )
            rstd16 = gtmp.tile([16, 1], F32)
            nc.vector.reciprocal(out=rstd16[:], in_=var16[:])
            bc2 = gtmp.tile([16, 2], F32)
            nc.gpsimd.tensor_copy(out=bc2[:, 0:1], in_=sg[:, 0:1])
            nc.gpsimd.tensor_copy(out=bc2[:, 1:2], in_=rstd16[:])
            # broadcast group values back to the 128 partitions of this block
            ps_bc = psum_mm.tile([128, 2], F32, tag="mm")
            nc.tensor.matmul(ps_bc[:], lhsT=Bm_sb[:], rhs=bc2[:], start=True, stop=True)
            mr = gtmp.tile([128, 2], F32)
            nc.vector.tensor_copy(out=mr[:], in_=ps_bc[:])
            nc.vector.tensor_mul(
                a_all[:, j : j + 1], mr[:, 1:2], vecs[:, GN_SCALE, j : j + 1]
            )
            tmpb = gtmp.tile([128, 1], F32)
            nc.vector.tensor_mul(tmpb[:], mr[:, 0:1], a_all[:, j : j + 1])
            nc.vector.tensor_sub(
                b_all[:, j : j + 1], vecs[:, GN_BIAS, j : j + 1], tmpb[:]
            )

        # ---- fold GN affine into QKV weights and biases ----
        wsc = {}
        for name in ("wq", "wk", "wv"):
            t = const.tile([128, 2, C], BF16, tag=f"{name}s")
            for j in (0, 1):
                nc.vector.tensor_scalar_mul(
                    out=t[:, j, :], in0=w_sb[name][:, j, :], scalar1=a_all[:, j : j + 1]
                )
            wsc[name] = t
        wo_bf = const.tile([128, 2, C], BF16)
        for j in (0, 1):
            nc.vector.tensor_copy(out=wo_bf[:, j, :], in_=w_sb["wo"][:, j, :])
        fold = {}
        for name, bidx in (("wq", BQ), ("wk", BK), ("wv", BV)):
            fb = const.tile([128, 2], F32, tag=f"{name}fb")
            for ob in (0, 1):
                ps = psum_mm.tile([128, 1], F32, tag="mm")
                for j in (0, 1):
                    nc.tensor.matmul(
                        ps[:],
                        lhsT=w_sb[name][:, j, osl(ob)],
                        rhs=b_all[:, j : j + 1],
                        start=(j == 0),
                        stop=(j == 1),
                    )
                nc.scalar.activation(
                    out=fb[:, ob : ob + 1],
                    in_=ps[:],
                    func=AF.Identity,
                    bias=vecs[:, bidx, ob : ob + 1],
                    scale=1.0,
                )
            fold[name] = fb
        bop = const.tile([128, 2], F32)
        for ob in (0, 1):
            ps = psum_mm.tile([128, 1], F32, tag="mm")
            for j in (0, 1):
                nc.tensor.matmul(
                    ps[:],
                    lhsT=w_sb["wo"][:, j, osl(ob)],
                    rhs=fold["wv"][:, j : j + 1],
                    start=(j == 0),
                    stop=(j == 1),
                )
            nc.scalar.activation(
                out=bop[:, ob : ob + 1],
                in_=ps[:],
                func=AF.Identity,
                bias=vecs[:, BO, ob : ob + 1],
                scale=1.0,
            )

        # ---- QKV projections (bf16 in, fp32 psum, bf16 out) ----
        q_sb = const.tile([128, 2, N], BF16)
        k_sb = const.tile([128, 2, N], BF16)
        v_sb = const.tile([128, 32, C], BF16)
        for name, dest, scale in (("wq", q_sb, SCALE), ("wk", k_sb, 1.0)):
            for ob in (0, 1):
                for i in range(NCH):
                    nsl = slice(i * NCHUNK, (i + 1) * NCHUNK)
                    ps = psum_mm.tile([128, NCHUNK], F32, tag="mm")
                    for j in (0, 1):
                        nc.tensor.matmul(
                            ps[:],
                            lhsT=wsc[name][:, j, osl(ob)],
                            rhs=x_bf[:, j, nsl],
                            start=(j == 0),
                            stop=(j == 1),
                        )
                    nc.scalar.activation(
                        out=dest[:, ob, nsl],
                        in_=ps[:],
                        func=AF.Identity,
                        bias=fold[name][:, ob : ob + 1],
                        scale=scale,
                    )
        # v^T: [m(part-block), c(free)] so it can serve as attention lhsT
        for mp in range(16):
            ps = psum_mm.tile([128, 2, C], F32, tag="mm")
            for s2 in (0, 1):
                mb = 2 * mp + s2
                for j in (0, 1):
                    nc.tensor.matmul(
                        ps[:, s2, :],
                        lhsT=x_bf[:, j, mb * 128 : (mb + 1) * 128],
                        rhs=wsc["wv"][:, j, :],
                        start=(j == 0),
                        stop=(j == 1),
                    )
            nc.scalar.copy(out=v_sb[:, 2 * mp : 2 * mp + 2, :], in_=ps[:])

        # ---- attention, one 512-wide n-chunk at a time ----
        def scores_chunk(i):
            nsl = slice(i * NCHUNK, (i + 1) * NCHUNK)
            pt = ptp.tile([128, 32, NCHUNK], BF16, tag="pt")
            for mp in range(16):
                ps = psum_st.tile([128, 2, NCHUNK], F32, tag="st")
                for s2 in (0, 1):
                    mb = 2 * mp + s2
                    for j in (0, 1):
                        nc.tensor.matmul(
                            ps[:, s2, :],
                            lhsT=k_sb[:, j, mb * 128 : (mb + 1) * 128],
                            rhs=q_sb[:, j, nsl],
                            start=(j == 0),
                            stop=(j == 1),
                        )
                nc.scalar.activation(
                    out=pt[:, 2 * mp : 2 * mp + 2, :], in_=ps[:], func=AF.Exp
                )
            return pt

        def post_chunk(i, pt):
            nsl = slice(i * NCHUNK, (i + 1) * NCHUNK)
            # softmax denominator: binary-tree reduce over the 32 m-blocks on
            # the DVE (contiguous bf16 adds run in the 2x packed mode; the
            # last two levels accumulate in fp32), then one ones-matmul for
            # the partition direction, which also broadcasts the sum to all
            # 128 partitions.
            t16 = gtmp.tile([128, 16, NCHUNK], BF16, tag="stage16")
            nc.vector.tensor_add(t16[:], pt[:, 0:16, :], pt[:, 16:32, :])
            nc.vector.tensor_add(t16[:, 0:8, :], t16[:, 0:8, :], t16[:, 8:16, :])
            nc.vector.tensor_add(t16[:, 0:4, :], t16[:, 0:4, :], t16[:, 4:8, :])
            t2 = gtmp.tile([128, 2, NCHUNK], F32, tag="t2")
            nc.vector.tensor_add(t2[:], t16[:, 0:2, :], t16[:, 2:4, :])
            rs = gtmp.tile([128, NCHUNK], F32, tag="rs")
            nc.vector.tensor_add(rs[:], t2[:, 0, :], t2[:, 1, :])
            zps = psum_mm.tile([128, NCHUNK], F32, tag="mm")
            nc.tensor.matmul(zps[:], lhsT=ones_f[:], rhs=rs[:], start=True, stop=True)
            rcp = work.tile([128, NCHUNK], F32, tag="rcp")
            nc.vector.reciprocal(out=rcp[:], in_=zps[:])
            attn = work.tile([128, 2, NCHUNK], BF16, tag="attn")
            for cb in (0, 1):
                ps = psum_mm.tile([128, NCHUNK], F32, tag="mm")
                for mb in range(32):
                    nc.tensor.matmul(
                        ps[:],
                        lhsT=v_sb[:, mb, osl(cb)],
                        rhs=pt[:, mb, :],
                        start=(mb == 0),
                        stop=(mb == 31),
                    )
                nc.vector.tensor_mul(attn[:, cb, :], ps[:], rcp[:])
            xres = work.tile([128, 2, NCHUNK], F32, tag="xres")
            for ob in (0, 1):
                nc.sync.dma_start(out=xres[:, ob, :], in_=x_d[osl(ob), nsl])
            ot = work.tile([128, 2, NCHUNK], F32, tag="ot")
            for ob in (0, 1):
                ps = psum_mm.tile([128, NCHUNK], F32, tag="mm")
                for j in (0, 1):
                    nc.tensor.matmul(
                        ps[:],
                        lhsT=wo_bf[:, j, osl(ob)],
                        rhs=attn[:, j, :],
                        start=(j == 0),
                        stop=(j == 1),
                    )
                nc.scalar.activation(
                    out=ot[:, ob, :],
                    in_=ps[:],
                    func=AF.Identity,
                    bias=bop[:, ob : ob + 1],
                    scale=1.0,
                )
            nc.vector.tensor_add(ot[:], ot[:], xres[:])
            for ob in (0, 1):
                nc.sync.dma_start(out=out_d[osl(ob), nsl], in_=ot[:, ob, :])

        prev = None
        for i in range(NCH):
            pt = scores_chunk(i)
            if prev is not None:
                post_chunk(i - 1, prev)
            prev = pt
        post_chunk(NCH - 1, prev)

    nc.compile()
    return nc


_NC_CACHE = None


def _get_nc():
    global _NC_CACHE
    if _NC_CACHE is None:
        _NC_CACHE = _build()
    return _NC_CACHE


def _host_inputs(x, gn_scale, gn_bias, wq, bq, wk, bk, wv, bv, wo, bo):
    x = np.asarray(x, dtype=np.float32)
    b = x.shape[0]
    assert x.shape == (b, C, H, W) and b == NCORES
    xs = np.ascontiguousarray(x.reshape(b, C, N))
    vecs = np.ascontiguousarray(
        np.stack(
            [
                np.asarray(v, np.float32).reshape(C)
                for v in (gn_scale, gn_bias, bq, bk, bv, bo)
            ]
        )
    )
    wt = {
        name: np.ascontiguousarray(np.asarray(w, np.float32).T)
        for name, w in (("wq", wq), ("wk", wk), ("wv", wv), ("wo", wo))
    }
    A = np.zeros((128, 16), np.float32)
    A[np.arange(128), np.arange(128) // 8] = 1.0 / 8.0
    Bm = np.zeros((16, 128), np.float32)
    Bm[np.arange(128) // 8, np.arange(128)] = 1.0
    return [
        {
            "x": xs[i],
            "wq_t": wt["wq"],
            "wk_t": wt["wk"],
            "wv_t": wt["wv"],
            "wo_t": wt["wo"],
            "vecs": vecs,
            "A": A,
            "Bm": Bm,
        }
        for i in range(NCORES)
    ]


def kernel(x, gn_scale, gn_bias, wq, bq, wk, bk, wv, bv, wo, bo):
    in_maps = _host_inputs(x, gn_scale, gn_bias, wq, bq, wk, bk, wv, bv, wo, bo)
    nc = _get_nc()
    res = run_bass_kernel_spmd(nc, in_maps, core_ids=list(range(NCORES)))
    out = np.stack([res.results[i]["out"] for i in range(NCORES)])
    return np.ascontiguousarray(out.reshape(NCORES, C, H, W))


# revision 32
# speedup vs baseline: 1.9624x; 1.2462x over previous
"""Trainium2 Bass kernel for nn_AttentionBlock (GroupNorm + 1x1-conv QKV
self-attention + residual), data-parallel over batch across 8 NeuronCores.

Per-core computation (one batch element, C=256 channels, N=64*64=4096 pixels):
  xn = GroupNorm(32, C)(x) * gn_scale + gn_bias      -- folded into QKV weights
  q/k/v = W @ xn + b                                 -- bf16 matmuls, fp32 psum
  S[n,m] = (q/16)[:,n] . k[:,m]; P = exp(S)          -- S^T kept [m_p, n_f]
  attn = (P^T v) / rowsum(P); out = x + Wo@attn + bo'

Layout notes: S^T is computed as [m(part), n(free)] so P^T tiles feed the
attention matmul directly (lhsT=v^T block, rhs=P^T block, contract m).
Rowsum of P uses a DVE strided reduce over m-blocks plus a GPSIMD
partition_all_reduce. The GroupNorm affine is folded algebraically into the
QKV weights (w' = w * a, bias' = w @ b + bias), so normalized x is never
materialized; the fp32 residual path streams x from DRAM.
"""

import sys

if "/opt/trn_rl_repo" not in sys.path:
    sys.path.insert(0, "/opt/trn_rl_repo")

from contextlib import ExitStack

import numpy as np

import concourse.tile as tile
from concourse import bacc, bass_isa, mybir
from concourse.bass_utils import run_bass_kernel_spmd

C = 256
H = W = 64
N = H * W  # 4096
GROUPS = 32
EPS = 1e-5
NCORES = 8
NCHUNK = 512  # n-tile width for the attention loop
NCH = N // NCHUNK  # 8
SCALE = 1.0 / 16.0  # 1/sqrt(C)
CBF = 1.0
ONES_COMP = 1.0

F32 = mybir.dt.float32
BF16 = mybir.dt.bfloat16
FP8 = mybir.dt.float8e4
DR = mybir.MatmulPerfMode.DoubleRow
LN2 = 0.6931471805599453
AF = mybir.ActivationFunctionType

# index of each packed per-channel vector in the "vecs" input
GN_SCALE, GN_BIAS, BQ, BK, BV, BO = range(6)


def _build():
    nc = bacc.Bacc("TRN2", target_bir_lowering=False, debug=False)

    x_d = nc.dram_tensor("x", [C, N], F32, kind="ExternalInput").ap()
    w_d = {
        name: nc.dram_tensor(f"{name}_t", [C, C], F32, kind="ExternalInput").ap()
        for name in ("wq", "wk", "wv", "wo")
    }
    vecs_d = nc.dram_tensor("vecs", [6, C], F32, kind="ExternalInput").ap()
    A_d = nc.dram_tensor("A", [128, 16], F32, kind="ExternalInput").ap()
    Bm_d = nc.dram_tensor("Bm", [16, 128], F32, kind="ExternalInput").ap()
    out_d = nc.dram_tensor("out", [C, N], F32, kind="ExternalOutput").ap()

    def osl(j):
        return slice(j * 128, (j + 1) * 128)

    with tile.TileContext(nc) as tc, ExitStack() as ctx:
        const = ctx.enter_context(tc.tile_pool(name="const", bufs=1))
        gtmp = ctx.enter_context(tc.tile_pool(name="gtmp", bufs=1))
        work = ctx.enter_context(tc.tile_pool(name="work", bufs=2))
        ptp = ctx.enter_context(tc.tile_pool(name="ptp", bufs=2))
        psum_st = ctx.enter_context(tc.tile_pool(name="pst", bufs=2, space="PSUM"))
        psum_mm = ctx.enter_context(tc.tile_pool(name="pmm", bufs=4, space="PSUM"))

        # ---- load inputs; stage x per c-block (fp32 for exact GN stats),
        # keep a bf16 copy resident for the matmuls ----
        w_sb = {}
        for name in ("wq", "wk", "wv", "wo"):
            t = const.tile([128, 2, C], F32, tag=name)
            for j in (0, 1):
                nc.sync.dma_start(out=t[:, j, :], in_=w_d[name][osl(j), :])
            w_sb[name] = t
        vecs = const.tile([128, 6, 2], F32)
        nc.sync.dma_start(out=vecs[:], in_=vecs_d.rearrange("v (j p) -> p v j", p=128))
        A_sb = const.tile([128, 16], F32)
        nc.sync.dma_start(out=A_sb[:], in_=A_d[:])
        Bm_sb = const.tile([16, 128], F32)
        nc.sync.dma_start(out=Bm_sb[:], in_=Bm_d[:])
        eps16 = const.tile([16, 1], F32)
        nc.vector.memset(eps16[:], EPS)
        ones_f8 = const.tile([128, 2, 128], FP8)
        nc.vector.memset(ones_f8[:], 1.0)
        negln2 = const.tile([128, 1], F32)
        nc.vector.memset(negln2[:], -5.0 * LN2)

        x_bf = const.tile([128, 2, N], BF16)
        a_all = const.tile([128, 2], F32)
        b_all = const.tile([128, 2], F32)
        for j in (0, 1):
            stage = gtmp.tile([128, N], F32, tag="stage16")
            for s in range(4):
                ssl = slice(s * 1024, (s + 1) * 1024)
                nc.sync.dma_start(out=stage[:, ssl], in_=x_d[osl(j), ssl])
            nc.vector.tensor_scalar_mul(out=x_bf[:, j, :], in0=stage[:], scalar1=CBF)
            # ---- GroupNorm statistics -> per-partition affine a, b ----
            stats = gtmp.tile([128, 8, 6], F32)
            for s in range(8):
                nc.vector.bn_stats(
                    out=stats[:, s, :], in_=stage[:, s * 512 : (s + 1) * 512]
                )
            mv = gtmp.tile([128, 2], F32)
            nc.vector.bn_aggr(out=mv[:], in_=stats[:])
            # t2 = (mean_p, var_p + mean_p^2)
            t2 = gtmp.tile([128, 2], F32)
            nc.gpsimd.tensor_copy(out=t2[:, 0:1], in_=mv[:, 0:1])
            nc.vector.tensor_mul(t2[:, 1:2], mv[:, 0:1], mv[:, 0:1])
            nc.vector.tensor_add(t2[:, 1:2], t2[:, 1:2], mv[:, 1:2])
            # group stats: (1/8) * sum over the 8 partitions of each group
            ps_g = psum_mm.tile([16, 2], F32, tag="mm")
            nc.tensor.matmul(ps_g[:], lhsT=A_sb[:], rhs=t2[:], start=True, stop=True)
            sg = gtmp.tile([16, 2], F32)
            nc.vector.tensor_copy(out=sg[:], in_=ps_g[:])
            var16 = gtmp.tile([16, 1], F32)
            nc.vector.tensor_mul(var16[:], sg[:, 0:1], sg[:, 0:1])
            nc.vector.tensor_sub(var16[:], sg[:, 1:2], var16[:])
            nc.scalar.activation(
                out=var16[:], in_=var16[:], func=AF.Sqrt, bias=eps16[:], scale=1.0
            )
            rstd16 = gtmp.tile([16, 1], F32)
            nc.vector.reciprocal(out=rstd16[:], in_=var16[:])
            bc2 = gtmp.tile([16, 2], F32)
            nc.gpsimd.tensor_copy(out=bc2[:, 0:1], in_=sg[:, 0:1])
            nc.gpsimd.tensor_copy(out=bc2[:, 1:2], in_=rstd16[:])
            # broadcast group values back to the 128 partitions of this block
            ps_bc = psum_mm.tile([128, 2], F32, tag="mm")
            nc.tensor.matmul(ps_bc[:], lhsT=Bm_sb[:], rhs=bc2[:], start=True, stop=True)
            mr = gtmp.tile([128, 2], F32)
            nc.vector.tensor_copy(out=mr[:], in_=ps_bc[:])
            nc.vector.tensor_mul(
                a_all[:, j : j + 1], mr[:, 1:2], vecs[:, GN_SCALE, j : j + 1]
            )
            tmpb = gtmp.tile([128, 1], F32)
            nc.vector.tensor_mul(tmpb[:], mr[:, 0:1], a_all[:, j : j + 1])
            nc.vector.tensor_sub(
                b_all[:, j : j + 1], vecs[:, GN_BIAS, j : j + 1], tmpb[:]
            )

        # ---- fold GN affine into QKV weights and biases ----
        wsc = {}
        for name in ("wq", "wk", "wv"):
            t = const.tile([128, 2, C], BF16, tag=f"{name}s")
            for j in (0, 1):
                nc.vector.tensor_scalar(
                    out=t[:, j, :],
                    in0=w_sb[name][:, j, :],
                    scalar1=a_all[:, j : j + 1],
                    scalar2=CBF,
                    op0=mybir.AluOpType.mult,
                    op1=mybir.AluOpType.mult,
                )
            wsc[name] = t
        wo_bf = const.tile([128, 2, C], BF16)
        for j in (0, 1):
            nc.vector.tensor_scalar_mul(out=wo_bf[:, j, :], in0=w_sb["wo"][:, j, :], scalar1=CBF)
        fold = {}
        for name, bidx in (("wq", BQ), ("wk", BK), ("wv", BV)):
            fb = const.tile([128, 2], F32, tag=f"{name}fb")
            for ob in (0, 1):
                ps = psum_mm.tile([128, 1], F32, tag="mm")
                for j in (0, 1):
                    nc.tensor.matmul(
                        ps[:],
                        lhsT=w_sb[name][:, j, osl(ob)],
                        rhs=b_all[:, j : j + 1],
                        start=(j == 0),
                        stop=(j == 1),
                    )
                nc.scalar.activation(
                    out=fb[:, ob : ob + 1],
                    in_=ps[:],
                    func=AF.Identity,
                    bias=vecs[:, bidx, ob : ob + 1],
                    scale=1.0,
                )
            if name == "wq":
                # q gets 1/sqrt(C) folded into its ACT-copy scale, which
                # multiplies only the psum; pre-scale the bias to match
                nc.vector.tensor_scalar_mul(out=fb[:], in0=fb[:], scalar1=SCALE)
            fold[name] = fb
        bop = const.tile([128, 2], F32)
        for ob in (0, 1):
            ps = psum_mm.tile([128, 1], F32, tag="mm")
            for j in (0, 1):
                nc.tensor.matmul(
                    ps[:],
                    lhsT=w_sb["wo"][:, j, osl(ob)],
                    rhs=fold["wv"][:, j : j + 1],
                    start=(j == 0),
                    stop=(j == 1),
                )
            nc.scalar.activation(
                out=bop[:, ob : ob + 1],
                in_=ps[:],
                func=AF.Identity,
                bias=vecs[:, BO, ob : ob + 1],
                scale=1.0,
            )

        # ---- QKV projections (bf16 in, fp32 psum, bf16 out) ----
        # k first (scores need all of k), then q (chunk 0 first), then v
        # (only needed once the first attention matmuls run). PSUM->SBUF
        # copies alternate between ScalarE and VectorE so neither serializes
        # the phase.
        q_sb = const.tile([128, 2, N], FP8)
        k_sb = const.tile([128, 2, N], FP8)
        v_sb = const.tile([128, 32, C], FP8)

        def qk_proj(name, dest, scale):
            for ob in (0, 1):
                for i2 in range(NCH // 2):
                    ps = psum_st.tile([128, 2, NCHUNK], F32, tag="st")
                    for half in (0, 1):
                        i = 2 * i2 + half
                        nsl = slice(i * NCHUNK, (i + 1) * NCHUNK)
                        for j in (0, 1):
                            nc.tensor.matmul(
                                ps[:, half, :],
                                lhsT=wsc[name][:, j, osl(ob)],
                                rhs=x_bf[:, j, nsl],
                                start=(j == 0),
                                stop=(j == 1),
                            )
                    nc.scalar.activation(
                        out=dest[:, ob, 2 * i2 * NCHUNK : (2 * i2 + 2) * NCHUNK],
                        in_=ps[:],
                        func=AF.Identity,
                        bias=fold[name][:, ob : ob + 1],
                        scale=scale,
                    )

        def v_proj():
            # v^T: [m(part-block), c(free)] so it can serve as attention lhsT
            for mp in range(8):
                ps = psum_st.tile([128, 4, C], F32, tag="st")
                for s2 in range(4):
                    mb = 4 * mp + s2
                    for j in (0, 1):
                        nc.tensor.matmul(
                            ps[:, s2, :],
                            lhsT=x_bf[:, j, mb * 128 : (mb + 1) * 128],
                            rhs=wsc["wv"][:, j, :],
                            start=(j == 0),
                            stop=(j == 1),
                        )
                nc.scalar.mul(out=v_sb[:, 4 * mp : 4 * mp + 4, :], in_=ps[:], mul=CBF)

        # ---- attention, one 512-wide n-chunk at a time ----
        def scores_chunk(i):
            nsl = slice(i * NCHUNK, (i + 1) * NCHUNK)
            pt = ptp.tile([128, 32, NCHUNK], FP8, tag="pt")
            for mp in range(16):
                ps = psum_st.tile([128, 2, NCHUNK], F32, tag="st")
                for s2 in (0, 1):
                    mb = 2 * mp + s2
                    nc.tensor.matmul(
                        ps[:, s2, :],
                        lhsT=k_sb[:, :, mb * 128 : (mb + 1) * 128],
                        rhs=q_sb[:, :, nsl],
                        start=True,
                        stop=True,
                        perf_mode=DR,
                    )
                nc.scalar.activation(
                    out=pt[:, 2 * mp : 2 * mp + 2, :],
                    in_=ps[:],
                    func=AF.Exp,
                    bias=negln2[:],
                )
            return pt

        def post_chunk(i, pt):
            nsl = slice(i * NCHUNK, (i + 1) * NCHUNK)
            # softmax denominator via fp8 DoubleRow ones-matmul: contracts
            # 256 m rows per pass with a stationary ones weight; every out
            # partition receives the full sum (reduce + broadcast in one).
            zps = psum_mm.tile([128, NCHUNK], F32, tag="mm")
            for pr in range(16):
                nc.tensor.matmul(
                    zps[:],
                    lhsT=ones_f8[:],
                    rhs=pt[:, 2 * pr : 2 * pr + 2, :],
                    start=(pr == 0),
                    stop=(pr == 15),
                    perf_mode=DR,
                )
            rcp = work.tile([128, NCHUNK], F32, tag="rcp")
            nc.vector.reciprocal(out=rcp[:], in_=zps[:])
            attn = work.tile([128, 2, NCHUNK], BF16, tag="attn")
            for cb in (0, 1):
                ps = psum_mm.tile([128, NCHUNK], F32, tag="mm")
                for pr in range(16):
                    nc.tensor.matmul(
                        ps[:],
                        lhsT=v_sb[:, 2 * pr : 2 * pr + 2, osl(cb)],
                        rhs=pt[:, 2 * pr : 2 * pr + 2, :],
                        start=(pr == 0),
                        stop=(pr == 15),
                        perf_mode=DR,
                    )
                nc.vector.tensor_mul(attn[:, cb, :], ps[:], rcp[:])
            xres = work.tile([128, 2, NCHUNK], F32, tag="xres")
            for ob in (0, 1):
                nc.sync.dma_start(out=xres[:, ob, :], in_=x_d[osl(ob), nsl])
            ot = work.tile([128, 2, NCHUNK], F32, tag="ot")
            for ob in (0, 1):
                ps = psum_mm.tile([128, NCHUNK], F32, tag="mm")
                for j in (0, 1):
                    nc.tensor.matmul(
                        ps[:],
                        lhsT=wo_bf[:, j, osl(ob)],
                        rhs=attn[:, j, :],
                        start=(j == 0),
                        stop=(j == 1),
                    )
                nc.vector.tensor_scalar_add(
                    out=ot[:, ob, :], in0=ps[:], scalar1=bop[:, ob : ob + 1]
                )
            nc.vector.tensor_add(ot[:], ot[:], xres[:])
            for ob in (0, 1):
                nc.sync.dma_start(out=out_d[osl(ob), nsl], in_=ot[:, ob, :])

        qk_proj("wq", q_sb, SCALE * CBF)
        qk_proj("wk", k_sb, CBF)
        prev = scores_chunk(0)
        v_proj()
        for i in range(1, NCH):
            pt = scores_chunk(i)
            post_chunk(i - 1, prev)
            prev = pt
        post_chunk(NCH - 1, prev)

    nc.compile()
    return nc


_NC_CACHE = None


def _get_nc():
    global _NC_CACHE
    if _NC_CACHE is None:
        _NC_CACHE = _build()
    return _NC_CACHE


def _host_inputs(x, gn_scale, gn_bias, wq, bq, wk, bk, wv, bv, wo, bo):
    x = np.asarray(x, dtype=np.float32)
    b = x.shape[0]
    assert x.shape == (b, C, H, W) and b == NCORES
    xs = np.ascontiguousarray(x.reshape(b, C, N))
    vecs = np.ascontiguousarray(
        np.stack(
            [
                np.asarray(v, np.float32).reshape(C)
                for v in (gn_scale, gn_bias, bq, bk, bv, bo)
            ]
        )
    )
    wt = {
        name: np.ascontiguousarray(np.asarray(w, np.float32).T)
        for name, w in (("wq", wq), ("wk", wk), ("wv", wv), ("wo", wo))
    }
    A = np.zeros((128, 16), np.float32)
    A[np.arange(128), np.arange(128) // 8] = 1.0 / 8.0
    Bm = np.zeros((16, 128), np.float32)
    Bm[np.arange(128) // 8, np.arange(128)] = 1.0
    return [
        {
            "x": xs[i],
            "wq_t": wt["wq"],
            "wk_t": wt["wk"],
            "wv_t": wt["wv"],
            "wo_t": wt["wo"],
            "vecs": vecs,
            "A": A,
            "Bm": Bm,
        }
        for i in range(NCORES)
    ]


def kernel(x, gn_scale, gn_bias, wq, bq, wk, bk, wv, bv, wo, bo):
    in_maps = _host_inputs(x, gn_scale, gn_bias, wq, bq, wk, bk, wv, bv, wo, bo)
    nc = _get_nc()
    res = run_bass_kernel_spmd(nc, in_maps, core_ids=list(range(NCORES)))
    out = np.stack([res.results[i]["out"] for i in range(NCORES)])
    return np.ascontiguousarray(out.reshape(NCORES, C, H, W))
